# revision 22
# baseline (speedup 1.0000x reference)
"""Trainium2 Bass kernel for nn_Attn (Bahdanau-style attention scores).

Reference computation:
    energy[s,b,:] = W @ enc[s,b,:] + bias          [S,B,H]
    scores[b,s]   = hidden[0,b,:] . energy[s,b,:]  [B,S]
    out           = softmax(scores, axis=-1)[:,None,:]

Key rewrite: scores[b,s] = (W^T hidden_b) . enc[s,b,:] + hidden_b . bias.
The second term is constant in s, so it is invariant under softmax and is
dropped entirely.  v_b = W^T hidden_b is a tiny [B, 2H] matvec done on the
tensor engine.

fp16 edition: enc, W, hidden and v are all fp16 (host-side cast), which
halves HBM traffic to ~36 MiB/core (hard floor ~110-115 us at ~330 GB/s).

The S*B*2H dot-product sweep is split BY BATCH across two pipelines so
every engine stays under the DMA streaming time:
  batches 0-1 (natural [s,b,k] layout, s on partitions):
      DVE tensor_mul fp16 (2x mode, ~1.6us/tile) +
      ScalarE activation-Copy accum (~1.7us/tile)      -> scores[s_p, b, t]
  batches 2-3 (host-TRANSPOSED [k,s] layout, k on partitions):
      TensorE matvec: psum[1, 512] += v_kc^T @ encT[kc, s-chunk]
      accumulated over the 16 k-chunks; lands directly in softmax layout.
Engine busy estimate: DVE ~60us, Scalar ~65us, PE ~60-95us, all < DMA.

Sharding: data-parallel over batch B (4 batch rows per core, 8 cores).
Softmax tail unchanged (fp32).
"""

import numpy as np

# Problem sizes (hardcoded per harness contract).
H = 1024          # hidden size
K = 2 * H         # 2H = contraction dim of W
S = 2048          # encoder sequence length
B = 32            # batch
N_CORES = 8
BPC = B // N_CORES  # batch rows per core = 4
NB_E = 1          # batches swept element-wise (DVE+Scalar)
NB_P = BPC - NB_E  # batches swept on the tensor engine: b = 2, 3

ST = 128          # s-tile (partition dim) for the element-wise sweep
KC = 512          # psum free chunk for the v matmul
NKC = K // KC     # 4
HC = 128          # h chunk (matmul contraction tile)
NHC = H // HC     # 8
NKP = K // 128    # 16 k-chunks of 128 (PE sweep contraction tiles)

_CACHE = {}


def _emit(ctx, tc, enc, enct, hidT, w, out):
    """Emit the per-core program.

    enc : DRAM [S, NB_E, K]  fp16           (batches 0-1, natural layout)
    enct: DRAM [NB_P, NSC, NKP, 128, SCW] fp16 (batches 2-3, transposed)
    hidT: DRAM [128, NHC*BPC] fp16, layout [p][c][b] for h = c*128 + p
    w   : DRAM [NKC, 128, NHC, KC] fp16 (wq[q][p][c][k] = W[c*128+p, q*KC+k])
    out : DRAM [BPC, S] fp32  (softmax probabilities)
    """
    from concourse import mybir
    from concourse.masks import make_identity

    nc = tc.nc
    f32 = mybir.dt.float32
    f16 = mybir.dt.float16

    NST = S // ST          # element-wise s-tiles (16 at full size)
    SCW = min(512, S)      # PE-sweep s-chunk width
    NSC = S // SCW         # PE-sweep s-chunks per batch

    singles = ctx.enter_context(tc.tile_pool(name="singles", bufs=1))
    wpool = ctx.enter_context(tc.tile_pool(name="wpool", bufs=2))
    encpool = ctx.enter_context(tc.tile_pool(name="encp", bufs=8))
    tencpool = ctx.enter_context(tc.tile_pool(name="tencp", bufs=4))
    prodpool = ctx.enter_context(tc.tile_pool(name="prodp", bufs=4))
    vpsum = ctx.enter_context(tc.tile_pool(name="vpsum", bufs=1, space="PSUM"))
    bcpsum = ctx.enter_context(tc.tile_pool(name="bcpsum", bufs=1, space="PSUM"))
    spsum = ctx.enter_context(tc.tile_pool(name="spsum", bufs=2, space="PSUM"))
    tpsum = ctx.enter_context(tc.tile_pool(name="tpsum", bufs=1, space="PSUM"))
    small = ctx.enter_context(tc.tile_pool(name="small", bufs=2))

    # ---- constants (no input deps; scheduled early) ---------------------
    ident = singles.tile([128, 128], f32)
    make_identity(nc, ident)
    ident16 = singles.tile([128, 128], f16)
    make_identity(nc, ident16)
    ones16 = singles.tile([1, 128], f16)
    nc.vector.memset(ones16, 1.0)

    # ---- PE warm-up ------------------------------------------------------
    # TensorE clocks at 1.2 GHz until it has been busy ~3us, then 2.4 GHz.
    # Burn dummy matmuls on a scratch PSUM bank while the W DMAs stream.
    warm_ps = bcpsum.tile([128, KC], f32, name="warm_ps", tag="bc_ps")
    for _ in range(24):
        nc.tensor.matmul(
            warm_ps[:, 0:128], lhsT=ident, rhs=ident, start=True, stop=True
        )

    # ---- load hidden^T (tiny, fp16) -------------------------------------
    hid_sb = singles.tile([128, NHC * BPC], f16)
    nc.scalar.dma_start(out=hid_sb, in_=hidT)

    # ---- v = W^T h, quarter-by-quarter over k ---------------------------
    # W streams as 4 column-quarter tiles [128, NHC, KC] fp16.  Per quarter:
    # matvec into psum -> v16_sb (fp16) -> flatten row -> for b 0-1 a PE
    # ones-matmul broadcast into v_bc; for b 2-3 a strided SBUF DMA into the
    # transposed vT_sb [k_p, kc, b] used as the PE-sweep stationary weights.
    v_bc = singles.tile([128, NB_E, K], f16)
    vT_sb = singles.tile([128, NKP, NB_P], f16)
    v16_sb = singles.tile([BPC, K], f16)
    w_dmas = []
    for q in range(NKC):
        w_sb = wpool.tile([128, NHC, KC], f16, name="w_sb", tag="w_sb")
        weng = nc.scalar if (q % 2 == 0) else nc.sync  # heads of both rings
        w_dmas.append(
            weng.dma_start(out=w_sb, in_=w[q])
        )
        v_ps = vpsum.tile([BPC, KC], f32, name="v_ps", tag="v_ps", bufs=1)
        for c in range(NHC):
            nc.tensor.matmul(
                v_ps[:, :],
                lhsT=hid_sb[:, c * BPC:(c + 1) * BPC],
                rhs=w_sb[:, c, :],
                start=(c == 0),
                stop=(c == NHC - 1),
            )
        # downcast to fp16 on the psum->sbuf copy
        nc.scalar.copy(out=v16_sb[:, q * KC:(q + 1) * KC], in_=v_ps[:, :])
        # flatten the 4 v rows of this quarter onto partition 0
        v_row = singles.tile([1, BPC * KC], f16, name="v_row", tag="v_row")
        nc.gpsimd.dma_start(out=v_row, in_=v16_sb[:, q * KC:(q + 1) * KC])
        ncc = KC // 128  # k-chunks of 128 in this quarter
        # v row order is [pe batches..., elementwise batches...] so the
        # transpose lhsT sits at base partition 0 (hw requirement).
        for b in range(NB_E):
            bc_ps = bcpsum.tile([128, KC], f32, name="bc_ps", tag="bc_ps")
            nc.tensor.matmul(
                bc_ps[:, :],
                lhsT=ones16,
                rhs=v_row[0:1, (NB_P + b) * KC:(NB_P + b + 1) * KC],
                start=True,
                stop=True,
            )
            eng = nc.vector if (q * BPC + b) % 2 == 0 else nc.scalar
            if eng is nc.vector:
                eng.tensor_copy(v_bc[:, b, q * KC:(q + 1) * KC], bc_ps[:, :])
            else:
                eng.copy(out=v_bc[:, b, q * KC:(q + 1) * KC], in_=bc_ps[:, :])
        # vT_sb[p, q*ncc + cc, :] = v_{NB_E+b'}[q*KC + cc*128 + p] via PE
        # transpose of the fp16 v rows (2x128 chunks -> psum [128, 2])
        for cc in range(ncc):
            tr_ps = tpsum.tile([128, NB_P], f16, name="tr_ps", tag="tr_ps",
                               bufs=1)
            nc.tensor.transpose(
                tr_ps[:, :],
                v16_sb[0:NB_P,
                       q * KC + cc * 128:q * KC + (cc + 1) * 128],
                ident16[0:NB_P, 0:NB_P],
            )
            eng = nc.vector if cc % 2 == 0 else nc.scalar
            if eng is nc.vector:
                eng.tensor_copy(vT_sb[:, q * ncc + cc, :], tr_ps[:, :])
            else:
                eng.copy(out=vT_sb[:, q * ncc + cc, :], in_=tr_ps[:, :])

    # ---- main sweep ------------------------------------------------------
    # Two interleaved streams share the sync DMA ring roughly in bandwidth
    # ratio (2 natural 1-MiB tiles : 1 transposed 2-MiB tile).
    scores = singles.tile([128, NB_E, NST], f32)
    s4 = singles.tile([NB_E, S], f32)
    # per-(batch, chunk) score tiles for the PE sweep (partition 0 each)
    sPc = [[singles.tile([1, SCW], f32, name=f"sPc{i}_{j}") for j in range(NSC)]
           for i in range(NB_P)]
    # per-batch online-softmax state: negated chunk maxes and chunk exp-sums
    nmx = [singles.tile([1, NSC], f32, name=f"nmx{i}") for i in range(NB_P)]
    rr = [singles.tile([1, NSC], f32, name=f"rr{i}") for i in range(NB_P)]

    def row_softmax(row, eng_r):
        """Softmax over the free axis of a [p, S] tile (element-wise batches)."""
        p = row.shape[0]
        nm = small.tile([p, 1], f32, name="nm", tag=f"nm{p}", bufs=2)
        eng_r.tensor_reduce(
            out=nm, in_=row, axis=mybir.AxisListType.X,
            op=mybir.AluOpType.max, negate=True,
        )
        r = small.tile([p, 1], f32, name="r", tag=f"r{p}", bufs=2)
        nc.scalar.activation(
            out=row, in_=row, func=mybir.ActivationFunctionType.Exp,
            bias=nm, scale=1.0, accum_out=r,
        )
        inv = small.tile([p, 1], f32, name="inv", tag=f"inv{p}", bufs=2)
        eng_r.reciprocal(inv, r)
        eng_r.tensor_scalar_mul(row, row, inv)

    # round-robin the bulk stream across both HWDGE rings so neither ring
    # head-of-line-blocks the stream and both start right behind the W
    # quarters already queued on them
    _ring_state = [0]

    def next_ring():
        _ring_state[0] ^= 1
        return nc.sync if _ring_state[0] else nc.scalar

    # generator for the PE-sweep (b, sc) units
    pe_units = [(b, sc) for b in range(NB_P) for sc in range(NSC)]
    pe_i = 0

    def emit_pe_unit():
        nonlocal pe_i
        if pe_i >= len(pe_units):
            return
        bp, sc = pe_units[pe_i]
        pe_i += 1
        te = tencpool.tile([128, NKP, SCW], f16, name="te", tag="te")
        next_ring().dma_start(out=te, in_=enct[bp, sc])
        chain = spsum.tile([1, SCW], f32, name="chain", tag="chain")
        for kc in range(NKP):
            nc.tensor.matmul(
                chain[:, :],
                lhsT=vT_sb[:, kc, bp:bp + 1],
                rhs=te[:, kc, :],
                start=(kc == 0),
                stop=(kc == NKP - 1),
            )
        # online softmax, chunk-local pass: m_sc (negated) and
        # e_sc = exp(x - m_sc) with running sum straight out of PSUM
        nc.vector.tensor_reduce(
            out=nmx[bp][0:1, sc:sc + 1], in_=chain[:, :],
            axis=mybir.AxisListType.X, op=mybir.AluOpType.max, negate=True,
        )
        nc.scalar.activation(
            out=sPc[bp][sc], in_=chain[:, :],
            func=mybir.ActivationFunctionType.Exp,
            bias=nmx[bp][0:1, sc:sc + 1], scale=1.0,
            accum_out=rr[bp][0:1, sc:sc + 1],
        )
        if sc == NSC - 1:
            # combine chunks: m = max_sc m_sc (nm_final = -m), rescale
            # factors f_sc = exp(m_sc - m), r = sum rr_sc * f_sc,
            # out_sc = e_sc * f_sc / r
            nmf = small.tile([1, 1], f32, name="nmf", tag="nmf", bufs=2)
            nc.vector.tensor_reduce(
                out=nmf, in_=nmx[bp], axis=mybir.AxisListType.X,
                op=mybir.AluOpType.min,
            )
            dd = small.tile([1, NSC], f32, name="dd", tag="dd", bufs=2)
            nc.vector.tensor_scalar_sub(dd, nmx[bp], nmf)
            ff = small.tile([1, NSC], f32, name="ff", tag="ff", bufs=2)
            nc.scalar.activation(
                out=ff, in_=dd, func=mybir.ActivationFunctionType.Exp,
                bias=0.0, scale=-1.0,
            )
            rf = small.tile([1, NSC], f32, name="rf", tag="rf", bufs=2)
            nc.vector.tensor_mul(rf, rr[bp], ff)
            rtot = small.tile([1, 1], f32, name="rtot", tag="rtot", bufs=2)
            nc.vector.tensor_reduce(
                out=rtot, in_=rf, axis=mybir.AxisListType.X,
                op=mybir.AluOpType.add,
            )
            inv = small.tile([1, 1], f32, name="pinv", tag="pinv", bufs=2)
            nc.vector.reciprocal(inv, rtot)
            gg = small.tile([1, NSC], f32, name="gg", tag="gg", bufs=2)
            nc.vector.tensor_scalar_mul(gg, ff, inv)
            for j in range(NSC):
                if j % 2 == 0:
                    nc.vector.tensor_scalar_mul(
                        sPc[bp][j], sPc[bp][j], gg[0:1, j:j + 1]
                    )
                else:
                    nc.scalar.mul(sPc[bp][j], sPc[bp][j], gg[0:1, j:j + 1])
                deng = nc.scalar if j % 2 == 0 else nc.sync
                deng.dma_start(
                    out=out[NB_E + bp:NB_E + bp + 1,
                            j * SCW:(j + 1) * SCW],
                    in_=sPc[bp][j],
                )

    # Stream order: W quarters were issued first on both rings (no holds
    # needed -- in-order rings drain W at full rate before enc).  Natural
    # tiles are front-loaded so the element-wise batch finishes ~85% in and
    # its long transpose-softmax tail hides under the remaining te stream;
    # the last te units are clustered to keep the PE continuously busy (at
    # full clock) through the end.
    for st in range(NST):
        enc_sb = encpool.tile([128, NB_E, K], f16)
        next_ring().dma_start(
            out=enc_sb,
            in_=enc[st * ST:(st + 1) * ST, :, :],
        )
        for bi in range(NB_E):
            prod = prodpool.tile([128, K], f16, name="prod", tag="prod")
            nc.vector.tensor_mul(prod, enc_sb[:, bi, :], v_bc[:, bi, :])
            nc.scalar.activation(
                out=prod,
                in_=prod,
                func=mybir.ActivationFunctionType.Copy,
                bias=0.0,
                scale=1.0,
                accum_out=scores[:, bi, st:st + 1],
            )
        if st % 2 == 1:
            emit_pe_unit()
    while pe_i < len(pe_units):
        emit_pe_unit()

    # ---- softmax for the element-wise batches ---------------------------
    # scores [128 s_in, (b t)] -> PE transpose -> [(b t), s_in] ->
    # SBUF->SBUF DMA reshape -> s4 [NB_E, S] -> free-axis softmax chain.
    # (PE-swept batches emitted their own chains inside the sweep.)
    sc2 = scores.rearrange("p b t -> p (b t)")
    scT_ps = tpsum.tile([NB_E * NST, 128], f32)
    nc.tensor.transpose(scT_ps[:, :], sc2, ident[:, :])
    scT = small.tile([NB_E * NST, 128], f32)
    nc.vector.tensor_copy(scT, scT_ps[:, :])
    nc.sync.dma_start(out=s4, in_=scT)

    row_softmax(s4, nc.vector)
    nc.sync.dma_start(out=out[0:NB_E, :], in_=s4)


def _declare(nc, S_=None):
    """Declare the per-core DRAM tensors (fp16 inputs, fp32 output)."""
    from concourse import mybir

    S_ = S if S_ is None else S_
    scw = min(512, S_)
    nsc = S_ // scw
    enc_d = nc.dram_tensor(
        "enc", [S_, NB_E, K], mybir.dt.float16, kind="ExternalInput"
    )
    enct_d = nc.dram_tensor(
        "enct", [NB_P, nsc, 128, NKP, scw], mybir.dt.float16,
        kind="ExternalInput",
    )
    hid_d = nc.dram_tensor(
        "hidT", [128, NHC * BPC], mybir.dt.float16, kind="ExternalInput"
    )
    w_d = nc.dram_tensor(
        "w", [NKC, 128, NHC, KC], mybir.dt.float16, kind="ExternalInput"
    )
    out_d = nc.dram_tensor(
        "attn_out", [BPC, S_], mybir.dt.float32, kind="ExternalOutput"
    )
    return enc_d, enct_d, hid_d, w_d, out_d


def _build():
    if "nc" in _CACHE:
        return _CACHE["nc"]
    from contextlib import ExitStack

    import concourse.bacc as bacc
    import concourse.tile as tile

    nc = bacc.Bacc(
        "TRN2", target_bir_lowering=False, debug=False, num_devices=N_CORES
    )
    enc_d, enct_d, hid_d, w_d, out_d = _declare(nc)

    with tile.TileContext(nc) as tc:
        with ExitStack() as ctx:
            _emit(
                ctx, tc, enc_d.ap(), enct_d.ap(), hid_d.ap(), w_d.ap(),
                out_d.ap(),
            )
    nc.compile()
    _CACHE["nc"] = nc
    return nc


def _make_core_inputs(hid_bpc, enc_bpc, w16):
    """hid_bpc [BPC, H], enc_bpc [S', BPC, K] fp16 -> core in_map (fp16)."""
    s_ = enc_bpc.shape[0]
    scw = min(512, s_)
    nsc = s_ // scw
    # batch order [pe batches (NB_E..), elementwise batches (0..NB_E-1)] so
    # the v rows for the PE sweep land at base partition 0.
    hid_perm = np.concatenate([hid_bpc[NB_E:], hid_bpc[:NB_E]], axis=0)
    hidT = np.ascontiguousarray(
        hid_perm.T.reshape(NHC, 128, BPC).transpose(1, 0, 2).reshape(128, NHC * BPC)
    ).astype(np.float16)
    enc_n = np.ascontiguousarray(enc_bpc[:, :NB_E, :], dtype=np.float16)
    # enct[b', sc, p, kc, s'] = enc[sc*scw + s', NB_E + b', kc*128 + p]
    enct = np.ascontiguousarray(
        enc_bpc[:, NB_E:, :]
        .reshape(nsc, scw, NB_P, NKP, 128)
        .transpose(2, 0, 4, 3, 1)
        .astype(np.float16)
    )
    return {"enc": enc_n, "enct": enct, "hidT": hidT, "w": w16}


def _make_in_maps(hidden, encoder_outputs, W):
    # wq[q][p][c][k] = W[c*128 + p, q*KC + k], contiguous per-quarter tiles
    w16 = np.ascontiguousarray(
        W.astype(np.float16).reshape(NHC, 128, NKC, KC).transpose(2, 1, 0, 3)
    )
    enc16 = encoder_outputs.astype(np.float16)
    in_maps = []
    for i in range(N_CORES):
        b0 = i * BPC
        in_maps.append(
            _make_core_inputs(
                hidden[0, b0:b0 + BPC, :], enc16[:, b0:b0 + BPC, :], w16
            )
        )
    return in_maps


def kernel(hidden, encoder_outputs, W, b):
    from concourse import bass_utils

    nc = _build()
    in_maps = _make_in_maps(
        np.asarray(hidden), np.asarray(encoder_outputs), np.asarray(W)
    )
    res = bass_utils.run_bass_kernel_spmd(
        nc, in_maps, core_ids=list(range(N_CORES))
    )
    out = np.concatenate(
        [res.results[i]["attn_out"] for i in range(N_CORES)], axis=0
    )  # [B, S]
    return out[:, None, :].astype(np.float32)


# revision 23
# speedup vs baseline: 1.0765x; 1.0765x over previous
"""Trainium2 Bass kernel for nn_Attn (Bahdanau-style attention scores).

Reference computation:
    energy[s,b,:] = W @ enc[s,b,:] + bias          [S,B,H]
    scores[b,s]   = hidden[0,b,:] . energy[s,b,:]  [B,S]
    out           = softmax(scores, axis=-1)[:,None,:]

Key rewrite: scores[b,s] = (W^T hidden_b) . enc[s,b,:] + hidden_b . bias.
The second term is constant in s, so it is invariant under softmax and is
dropped entirely.  v_b = W^T hidden_b is a tiny [B, 2H] matvec done on the
tensor engine.

fp16 edition: enc, W, hidden and v are all fp16 (host-side cast), which
halves HBM traffic to ~36 MiB/core (hard floor ~110-115 us at ~330 GB/s).

The S*B*2H dot-product sweep is split BY BATCH across two pipelines so
every engine stays under the DMA streaming time:
  batches 0-1 (natural [s,b,k] layout, s on partitions):
      DVE tensor_mul fp16 (2x mode, ~1.6us/tile) +
      ScalarE activation-Copy accum (~1.7us/tile)      -> scores[s_p, b, t]
  batches 2-3 (host-TRANSPOSED [k,s] layout, k on partitions):
      TensorE matvec: psum[1, 512] += v_kc^T @ encT[kc, s-chunk]
      accumulated over the 16 k-chunks; lands directly in softmax layout.
Engine busy estimate: DVE ~60us, Scalar ~65us, PE ~60-95us, all < DMA.

Sharding: data-parallel over batch B (4 batch rows per core, 8 cores).
Softmax tail unchanged (fp32).
"""

import numpy as np

# Problem sizes (hardcoded per harness contract).
H = 1024          # hidden size
K = 2 * H         # 2H = contraction dim of W
S = 2048          # encoder sequence length
B = 32            # batch
N_CORES = 8
BPC = B // N_CORES  # batch rows per core = 4
NB_E = 1          # batches swept element-wise (DVE+Scalar)
NB_P = BPC - NB_E  # batches swept on the tensor engine: b = 2, 3

ST = 128          # s-tile (partition dim) for the element-wise sweep
KC = 512          # psum free chunk for the v matmul
NKC = K // KC     # 4
HC = 128          # h chunk (matmul contraction tile)
NHC = H // HC     # 8
NKP = K // 128    # 16 k-chunks of 128 (PE sweep contraction tiles)

_CACHE = {}


def _emit(ctx, tc, enc, enct, hidT, w, out):
    """Emit the per-core program.

    enc : DRAM [S, NB_E, K]  fp16           (batches 0-1, natural layout)
    enct: DRAM [NB_P, NSC, NKP, 128, SCW] fp16 (batches 2-3, transposed)
    hidT: DRAM [128, NHC*BPC] fp16, layout [p][c][b] for h = c*128 + p
    w   : DRAM [NKC, 128, NHC, KC] fp16 (wq[q][p][c][k] = W[c*128+p, q*KC+k])
    out : DRAM [BPC, S] fp32  (softmax probabilities)
    """
    from concourse import mybir
    from concourse.masks import make_identity

    nc = tc.nc
    f32 = mybir.dt.float32
    f16 = mybir.dt.float16

    NST = S // ST          # element-wise s-tiles (16 at full size)
    SCW = min(512, S)      # PE-sweep s-chunk width
    NSC = S // SCW         # PE-sweep s-chunks per batch

    singles = ctx.enter_context(tc.tile_pool(name="singles", bufs=1))
    wpool = ctx.enter_context(tc.tile_pool(name="wpool", bufs=2))
    encpool = ctx.enter_context(tc.tile_pool(name="encp", bufs=8))
    tencpool = ctx.enter_context(tc.tile_pool(name="tencp", bufs=4))
    prodpool = ctx.enter_context(tc.tile_pool(name="prodp", bufs=4))
    vpsum = ctx.enter_context(tc.tile_pool(name="vpsum", bufs=1, space="PSUM"))
    bcpsum = ctx.enter_context(tc.tile_pool(name="bcpsum", bufs=1, space="PSUM"))
    spsum = ctx.enter_context(tc.tile_pool(name="spsum", bufs=4, space="PSUM"))
    tpsum = ctx.enter_context(tc.tile_pool(name="tpsum", bufs=1, space="PSUM"))
    small = ctx.enter_context(tc.tile_pool(name="small", bufs=2))

    # ---- constants (no input deps; scheduled early) ---------------------
    ident = singles.tile([128, 128], f32)
    make_identity(nc, ident)
    ident16 = singles.tile([128, 128], f16)
    make_identity(nc, ident16)
    ones16 = singles.tile([1, 128], f16)
    nc.vector.memset(ones16, 1.0)

    # ---- PE warm-up ------------------------------------------------------
    # TensorE clocks at 1.2 GHz until it has been busy ~3us, then 2.4 GHz.
    # Burn dummy matmuls on a scratch PSUM bank while the W DMAs stream.
    warm_ps = bcpsum.tile([128, KC], f32, name="warm_ps", tag="bc_ps")
    for _ in range(24):
        nc.tensor.matmul(
            warm_ps[:, 0:128], lhsT=ident, rhs=ident, start=True, stop=True
        )

    # ---- load hidden^T (tiny, fp16) -------------------------------------
    hid_sb = singles.tile([128, NHC * BPC], f16)
    nc.scalar.dma_start(out=hid_sb, in_=hidT)

    # ---- v = W^T h, quarter-by-quarter over k ---------------------------
    # W streams as 4 column-quarter tiles [128, NHC, KC] fp16.  Per quarter:
    # matvec into psum -> v16_sb (fp16) -> flatten row -> for b 0-1 a PE
    # ones-matmul broadcast into v_bc; for b 2-3 a strided SBUF DMA into the
    # transposed vT_sb [k_p, kc, b] used as the PE-sweep stationary weights.
    v_bc = singles.tile([128, NB_E, K], f16)
    vT_sb = singles.tile([128, NKP, NB_P], f16)
    v16_sb = singles.tile([BPC, K], f16)
    w_dmas = []
    for q in range(NKC):
        w_sb = wpool.tile([128, NHC, KC], f16, name="w_sb", tag="w_sb")
        weng = nc.scalar if (q % 2 == 0) else nc.sync  # heads of both rings
        w_dmas.append(
            weng.dma_start(out=w_sb, in_=w[q])
        )
        v_ps = vpsum.tile([BPC, KC], f32, name="v_ps", tag="v_ps", bufs=1)
        for c in range(NHC):
            nc.tensor.matmul(
                v_ps[:, :],
                lhsT=hid_sb[:, c * BPC:(c + 1) * BPC],
                rhs=w_sb[:, c, :],
                start=(c == 0),
                stop=(c == NHC - 1),
            )
        # downcast to fp16 on the psum->sbuf copy
        nc.scalar.copy(out=v16_sb[:, q * KC:(q + 1) * KC], in_=v_ps[:, :])
        # flatten the 4 v rows of this quarter onto partition 0
        v_row = singles.tile([1, BPC * KC], f16, name="v_row", tag="v_row")
        nc.gpsimd.dma_start(out=v_row, in_=v16_sb[:, q * KC:(q + 1) * KC])
        ncc = KC // 128  # k-chunks of 128 in this quarter
        # v row order is [pe batches..., elementwise batches...] so the
        # transpose lhsT sits at base partition 0 (hw requirement).
        for b in range(NB_E):
            bc_ps = bcpsum.tile([128, KC], f32, name="bc_ps", tag="bc_ps")
            nc.tensor.matmul(
                bc_ps[:, :],
                lhsT=ones16,
                rhs=v_row[0:1, (NB_P + b) * KC:(NB_P + b + 1) * KC],
                start=True,
                stop=True,
            )
            eng = nc.vector if (q * BPC + b) % 2 == 0 else nc.scalar
            if eng is nc.vector:
                eng.tensor_copy(v_bc[:, b, q * KC:(q + 1) * KC], bc_ps[:, :])
            else:
                eng.copy(out=v_bc[:, b, q * KC:(q + 1) * KC], in_=bc_ps[:, :])
        # vT_sb[p, q*ncc + cc, :] = v_{NB_E+b'}[q*KC + cc*128 + p] via PE
        # transpose of the fp16 v rows (2x128 chunks -> psum [128, 2])
        for cc in range(ncc):
            tr_ps = tpsum.tile([128, NB_P], f16, name="tr_ps", tag="tr_ps",
                               bufs=1)
            nc.tensor.transpose(
                tr_ps[:, :],
                v16_sb[0:NB_P,
                       q * KC + cc * 128:q * KC + (cc + 1) * 128],
                ident16[0:NB_P, 0:NB_P],
            )
            eng = nc.vector if cc % 2 == 0 else nc.scalar
            if eng is nc.vector:
                eng.tensor_copy(vT_sb[:, q * ncc + cc, :], tr_ps[:, :])
            else:
                eng.copy(out=vT_sb[:, q * ncc + cc, :], in_=tr_ps[:, :])

    # ---- main sweep ------------------------------------------------------
    # Two interleaved streams share the sync DMA ring roughly in bandwidth
    # ratio (2 natural 1-MiB tiles : 1 transposed 2-MiB tile).
    scores = singles.tile([128, NB_E, NST], f32)
    s4 = singles.tile([NB_E, S], f32)
    # per-(batch, chunk) score tiles for the PE sweep (partition 0 each)
    sPc = [[singles.tile([1, SCW], f32, name=f"sPc{i}_{j}") for j in range(NSC)]
           for i in range(NB_P)]
    # per-batch online-softmax state: negated chunk maxes and chunk exp-sums
    nmx = [singles.tile([1, NSC], f32, name=f"nmx{i}") for i in range(NB_P)]
    rr = [singles.tile([1, NSC], f32, name=f"rr{i}") for i in range(NB_P)]

    def row_softmax(row, eng_r):
        """Softmax over the free axis of a [p, S] tile (element-wise batches)."""
        p = row.shape[0]
        nm = small.tile([p, 1], f32, name="nm", tag=f"nm{p}", bufs=2)
        eng_r.tensor_reduce(
            out=nm, in_=row, axis=mybir.AxisListType.X,
            op=mybir.AluOpType.max, negate=True,
        )
        r = small.tile([p, 1], f32, name="r", tag=f"r{p}", bufs=2)
        nc.scalar.activation(
            out=row, in_=row, func=mybir.ActivationFunctionType.Exp,
            bias=nm, scale=1.0, accum_out=r,
        )
        inv = small.tile([p, 1], f32, name="inv", tag=f"inv{p}", bufs=2)
        eng_r.reciprocal(inv, r)
        eng_r.tensor_scalar_mul(row, row, inv)

    # round-robin the bulk stream across both HWDGE rings so neither ring
    # head-of-line-blocks the stream and both start right behind the W
    # quarters already queued on them
    def next_ring():
        return nc.sync

    # generator for the PE-sweep (b, sc) units
    pe_units = [(b, sc) for b in range(NB_P) for sc in range(NSC)]
    pe_i = 0

    def emit_pe_unit():
        nonlocal pe_i
        if pe_i >= len(pe_units):
            return
        bp, sc = pe_units[pe_i]
        pe_i += 1
        te = tencpool.tile([128, NKP, SCW], f16, name="te", tag="te")
        next_ring().dma_start(out=te, in_=enct[bp, sc])
        chain = spsum.tile([1, SCW], f32, name="chain", tag="chain")
        for kc in range(NKP):
            nc.tensor.matmul(
                chain[:, :],
                lhsT=vT_sb[:, kc, bp:bp + 1],
                rhs=te[:, kc, :],
                start=(kc == 0),
                stop=(kc == NKP - 1),
            )
        # online softmax, chunk-local pass: m_sc (negated) and
        # e_sc = exp(x - m_sc) with running sum straight out of PSUM
        nc.vector.tensor_reduce(
            out=nmx[bp][0:1, sc:sc + 1], in_=chain[:, :],
            axis=mybir.AxisListType.X, op=mybir.AluOpType.max, negate=True,
        )
        nc.scalar.activation(
            out=sPc[bp][sc], in_=chain[:, :],
            func=mybir.ActivationFunctionType.Exp,
            bias=nmx[bp][0:1, sc:sc + 1], scale=1.0,
            accum_out=rr[bp][0:1, sc:sc + 1],
        )
        if sc == NSC - 1:
            # combine chunks: m = max_sc m_sc (nm_final = -m), rescale
            # factors f_sc = exp(m_sc - m), r = sum rr_sc * f_sc,
            # out_sc = e_sc * f_sc / r
            nmf = small.tile([1, 1], f32, name="nmf", tag="nmf", bufs=2)
            nc.vector.tensor_reduce(
                out=nmf, in_=nmx[bp], axis=mybir.AxisListType.X,
                op=mybir.AluOpType.min,
            )
            dd = small.tile([1, NSC], f32, name="dd", tag="dd", bufs=2)
            nc.vector.tensor_scalar_sub(dd, nmx[bp], nmf)
            ff = small.tile([1, NSC], f32, name="ff", tag="ff", bufs=2)
            nc.scalar.activation(
                out=ff, in_=dd, func=mybir.ActivationFunctionType.Exp,
                bias=0.0, scale=-1.0,
            )
            rf = small.tile([1, NSC], f32, name="rf", tag="rf", bufs=2)
            nc.vector.tensor_mul(rf, rr[bp], ff)
            rtot = small.tile([1, 1], f32, name="rtot", tag="rtot", bufs=2)
            nc.vector.tensor_reduce(
                out=rtot, in_=rf, axis=mybir.AxisListType.X,
                op=mybir.AluOpType.add,
            )
            inv = small.tile([1, 1], f32, name="pinv", tag="pinv", bufs=2)
            nc.vector.reciprocal(inv, rtot)
            gg = small.tile([1, NSC], f32, name="gg", tag="gg", bufs=2)
            nc.vector.tensor_scalar_mul(gg, ff, inv)
            for j in range(NSC):
                if j % 2 == 0:
                    nc.vector.tensor_scalar_mul(
                        sPc[bp][j], sPc[bp][j], gg[0:1, j:j + 1]
                    )
                else:
                    nc.scalar.mul(sPc[bp][j], sPc[bp][j], gg[0:1, j:j + 1])
                deng = nc.scalar if j % 2 == 0 else nc.sync
                deng.dma_start(
                    out=out[NB_E + bp:NB_E + bp + 1,
                            j * SCW:(j + 1) * SCW],
                    in_=sPc[bp][j],
                )

    # Stream order: W quarters were issued first on both rings (no holds
    # needed -- in-order rings drain W at full rate before enc).  Natural
    # tiles are front-loaded so the element-wise batch finishes ~85% in and
    # its long transpose-softmax tail hides under the remaining te stream;
    # the last te units are clustered to keep the PE continuously busy (at
    # full clock) through the end.
    for st in range(NST):
        enc_sb = encpool.tile([128, NB_E, K], f16)
        next_ring().dma_start(
            out=enc_sb,
            in_=enc[st * ST:(st + 1) * ST, :, :],
        )
        for bi in range(NB_E):
            prod = prodpool.tile([128, K], f16, name="prod", tag="prod")
            nc.vector.tensor_mul(prod, enc_sb[:, bi, :], v_bc[:, bi, :])
            nc.scalar.activation(
                out=prod,
                in_=prod,
                func=mybir.ActivationFunctionType.Copy,
                bias=0.0,
                scale=1.0,
                accum_out=scores[:, bi, st:st + 1],
            )
        if st % 2 == 1:
            emit_pe_unit()
    while pe_i < len(pe_units):
        emit_pe_unit()

    # ---- softmax for the element-wise batches ---------------------------
    # scores [128 s_in, (b t)] -> PE transpose -> [(b t), s_in] ->
    # SBUF->SBUF DMA reshape -> s4 [NB_E, S] -> free-axis softmax chain.
    # (PE-swept batches emitted their own chains inside the sweep.)
    sc2 = scores.rearrange("p b t -> p (b t)")
    scT_ps = tpsum.tile([NB_E * NST, 128], f32)
    nc.tensor.transpose(scT_ps[:, :], sc2, ident[:, :])
    scT = small.tile([NB_E * NST, 128], f32)
    nc.vector.tensor_copy(scT, scT_ps[:, :])
    nc.sync.dma_start(out=s4, in_=scT)

    row_softmax(s4, nc.vector)
    nc.sync.dma_start(out=out[0:NB_E, :], in_=s4)


def _declare(nc, S_=None):
    """Declare the per-core DRAM tensors (fp16 inputs, fp32 output)."""
    from concourse import mybir

    S_ = S if S_ is None else S_
    scw = min(512, S_)
    nsc = S_ // scw
    enc_d = nc.dram_tensor(
        "enc", [S_, NB_E, K], mybir.dt.float16, kind="ExternalInput"
    )
    enct_d = nc.dram_tensor(
        "enct", [NB_P, nsc, 128, NKP, scw], mybir.dt.float16,
        kind="ExternalInput",
    )
    hid_d = nc.dram_tensor(
        "hidT", [128, NHC * BPC], mybir.dt.float16, kind="ExternalInput"
    )
    w_d = nc.dram_tensor(
        "w", [NKC, 128, NHC, KC], mybir.dt.float16, kind="ExternalInput"
    )
    out_d = nc.dram_tensor(
        "attn_out", [BPC, S_], mybir.dt.float32, kind="ExternalOutput"
    )
    return enc_d, enct_d, hid_d, w_d, out_d


def _build():
    if "nc" in _CACHE:
        return _CACHE["nc"]
    from contextlib import ExitStack

    import concourse.bacc as bacc
    import concourse.tile as tile

    nc = bacc.Bacc(
        "TRN2", target_bir_lowering=False, debug=False, num_devices=N_CORES
    )
    enc_d, enct_d, hid_d, w_d, out_d = _declare(nc)

    with tile.TileContext(nc) as tc:
        with ExitStack() as ctx:
            _emit(
                ctx, tc, enc_d.ap(), enct_d.ap(), hid_d.ap(), w_d.ap(),
                out_d.ap(),
            )
    nc.compile()
    _CACHE["nc"] = nc
    return nc


def _make_core_inputs(hid_bpc, enc_bpc, w16):
    """hid_bpc [BPC, H], enc_bpc [S', BPC, K] fp16 -> core in_map (fp16)."""
    s_ = enc_bpc.shape[0]
    scw = min(512, s_)
    nsc = s_ // scw
    # batch order [pe batches (NB_E..), elementwise batches (0..NB_E-1)] so
    # the v rows for the PE sweep land at base partition 0.
    hid_perm = np.concatenate([hid_bpc[NB_E:], hid_bpc[:NB_E]], axis=0)
    hidT = np.ascontiguousarray(
        hid_perm.T.reshape(NHC, 128, BPC).transpose(1, 0, 2).reshape(128, NHC * BPC)
    ).astype(np.float16)
    enc_n = np.ascontiguousarray(enc_bpc[:, :NB_E, :], dtype=np.float16)
    # enct[b', sc, p, kc, s'] = enc[sc*scw + s', NB_E + b', kc*128 + p]
    enct = np.ascontiguousarray(
        enc_bpc[:, NB_E:, :]
        .reshape(nsc, scw, NB_P, NKP, 128)
        .transpose(2, 0, 4, 3, 1)
        .astype(np.float16)
    )
    return {"enc": enc_n, "enct": enct, "hidT": hidT, "w": w16}


def _make_in_maps(hidden, encoder_outputs, W):
    # wq[q][p][c][k] = W[c*128 + p, q*KC + k], contiguous per-quarter tiles
    w16 = np.ascontiguousarray(
        W.astype(np.float16).reshape(NHC, 128, NKC, KC).transpose(2, 1, 0, 3)
    )
    enc16 = encoder_outputs.astype(np.float16)
    in_maps = []
    for i in range(N_CORES):
        b0 = i * BPC
        in_maps.append(
            _make_core_inputs(
                hidden[0, b0:b0 + BPC, :], enc16[:, b0:b0 + BPC, :], w16
            )
        )
    return in_maps


def kernel(hidden, encoder_outputs, W, b):
    from concourse import bass_utils

    nc = _build()
    in_maps = _make_in_maps(
        np.asarray(hidden), np.asarray(encoder_outputs), np.asarray(W)
    )
    res = bass_utils.run_bass_kernel_spmd(
        nc, in_maps, core_ids=list(range(N_CORES))
    )
    out = np.concatenate(
        [res.results[i]["attn_out"] for i in range(N_CORES)], axis=0
    )  # [B, S]
    return out[:, None, :].astype(np.float32)
